# revision 1
# baseline (speedup 1.0000x reference)
"""MemNN (embedding_lookup) Trainium2 Bass kernel.

Strategy (8 NeuronCores, one NEFF, SPMD):
  - Data-parallel hops: batch dim sharded 8 ways (8 batches/core).
  - Host packs the 4 embedding tables interleaved per vocab row
    ([A0|A1|A2|A3][v], bf16) and, per core, compacts it to the core's
    unique vocab rows so indices fit dma_gather's int16 (~22.6K < 32767).
  - dma_gather streams all (story + query) embedding rows; PE matmuls
    with fixed block weights reduce each 128-row tile into per-sentence
    partial sums (one PSUM bank per gather chunk). Position encoding is
    rank-2 separable:
        pe[j,d] = a(j) + b(j) * k'(d),  a=1-j/J, b=2j/J-1, k'=d/D
    so m = S_a + k' * S_b needs only two weighted sums per sentence.
  - 3 attention hops on PE/DVE/ACT in transposed [d, sentence] layout.
  - AllGather u across cores; vocab-sharded logits z = u @ A3^T using a
    host-pre-transposed A3 shard; log_softmax via AllReduce of exp-sums.
"""

import numpy as np
import ml_dtypes

import concourse.bass as bass
import concourse.mybir as mybir
import concourse.tile as tile
from concourse import bacc
import concourse.bass_utils as bass_utils

F32 = mybir.dt.float32
AF = mybir.ActivationFunctionType
ALU = mybir.AluOpType
AX = mybir.AxisListType

NEG = -1e30


class Cfg:
    def __init__(self, ncore=8, B=64, S=50, J=64, QW=16, V=100000, E=128,
                 ucap=24576, gchunk=1024, use_bf16=True, z_f32=False):
        self.ncore, self.B, self.S, self.J, self.QW = ncore, B, S, J, QW
        self.V, self.E, self.ucap, self.gchunk = V, E, ucap, gchunk
        self.use_bf16, self.z_f32 = use_bf16, z_f32
        self.Bc = B // ncore
        self.NS = self.Bc * S              # sentences per core
        self.NX = self.NS * J              # story rows per core
        self.NQ = self.Bc * QW             # query rows per core
        assert self.NQ == 128 and self.NX % 128 == 0
        self.NPOS = self.NX + self.NQ
        self.xtiles = self.NX // 128
        self.SPT = 128 // J                # sentences per 128-row tile
        assert 128 % J == 0
        self.VSH = V // ncore
        sizes = []
        rem = self.NPOS
        while rem > 0:
            s = min(gchunk, rem)
            sizes.append(s)
            rem -= s
        assert all(s % 128 == 0 for s in sizes)
        self.gsizes = sizes
        self.DT = mybir.dt.bfloat16 if use_bf16 else mybir.dt.float32
        self.npdt = ml_dtypes.bfloat16 if use_bf16 else np.float32
        self.zdt = F32 if z_f32 else self.DT
        self.znp = np.float32 if z_f32 else self.npdt

    def key(self):
        return (self.ncore, self.B, self.S, self.J, self.QW, self.V, self.E,
                self.ucap, self.gchunk, self.use_bf16, self.z_f32)


def build_module(cfg):
    c = cfg
    E, NS, Bc, VSH, S = c.E, c.NS, c.Bc, c.VSH, c.S
    DT = c.DT
    nc = bacc.Bacc("TRN2", target_bir_lowering=False, debug=False,
                   num_devices=c.ncore, num_swdge_queues=1)

    t_tab = nc.dram_tensor("tabc", [c.ucap, 4 * E], DT, kind="ExternalInput")
    nch = len(c.gsizes)
    t_idx = nc.dram_tensor("idx", [nch, 128, c.gchunk // 16], mybir.dt.int16,
                           kind="ExternalInput")
    t_a3t = nc.dram_tensor("a3t", [E, VSH], c.zdt, kind="ExternalInput")
    t_wab = nc.dram_tensor("wab", [128, 2 * c.SPT], DT, kind="ExternalInput")
    t_wabc = nc.dram_tensor("wabc", [128, 3 * c.SPT], DT, kind="ExternalInput")
    t_wc = nc.dram_tensor("wc", [128, c.SPT], DT, kind="ExternalInput")
    t_wq = nc.dram_tensor("wq", [128, Bc], DT, kind="ExternalInput")
    t_tat = nc.dram_tensor("tat", [E, S], F32, kind="ExternalInput")
    t_tct = nc.dram_tensor("tct", [E, S], F32, kind="ExternalInput")
    t_kp = nc.dram_tensor("kp", [E, 1], F32, kind="ExternalInput")
    t_mask = nc.dram_tensor("mask", [Bc, NS], F32, kind="ExternalInput")
    t_id128 = nc.dram_tensor("id128", [128, 128], F32, kind="ExternalInput")
    t_idq = nc.dram_tensor("idq", [Bc, Bc], F32, kind="ExternalInput")

    t_o = nc.dram_tensor("o", [c.B, VSH], F32, kind="ExternalOutput")
    t_du = nc.dram_tensor("du", [E, c.B], F32, kind="ExternalOutput")

    with tile.TileContext(nc) as tc:
        with tc.tile_pool(name="const", bufs=1) as cpool, \
             tc.tile_pool(name="gp", bufs=3) as gpool, \
             tc.tile_pool(name="wk", bufs=1) as wk, \
             tc.tile_pool(name="hp", bufs=2) as hp, \
             tc.tile_pool(name="big", bufs=1) as big, \
             tc.tile_pool(name="psG", bufs=2, space="PSUM") as psG, \
             tc.tile_pool(name="psH", bufs=1, space="PSUM") as psH, \
             tc.tile_pool(name="psZ", bufs=2, space="PSUM") as psZ, \
             tc.tile_pool(name="dram", bufs=1, space="DRAM") as dram:

            # ---- constant loads -------------------------------------------
            wab = cpool.tile([128, 2 * c.SPT], DT)
            nc.sync.dma_start(out=wab[:], in_=t_wab.ap())
            wabc = cpool.tile([128, 3 * c.SPT], DT)
            nc.sync.dma_start(out=wabc[:], in_=t_wabc.ap())
            wc_t = cpool.tile([128, c.SPT], DT)
            nc.sync.dma_start(out=wc_t[:], in_=t_wc.ap())
            wq = cpool.tile([128, Bc], DT)
            nc.sync.dma_start(out=wq[:], in_=t_wq.ap())
            tat = cpool.tile([E, S], F32)
            nc.sync.dma_start(out=tat[:], in_=t_tat.ap())
            tct = cpool.tile([E, S], F32)
            nc.sync.dma_start(out=tct[:], in_=t_tct.ap())
            kp = cpool.tile([E, 1], F32)
            nc.sync.dma_start(out=kp[:], in_=t_kp.ap())
            mask = cpool.tile([Bc, NS], F32)
            nc.sync.dma_start(out=mask[:], in_=t_mask.ap())
            id128 = cpool.tile([128, 128], F32)
            nc.sync.dma_start(out=id128[:], in_=t_id128.ap())
            idq = cpool.tile([Bc, Bc], F32)
            nc.sync.dma_start(out=idq[:], in_=t_idq.ap())
            idxs = cpool.tile([128, nch, c.gchunk // 16], mybir.dt.int16)
            for g in range(nch):
                nc.sync.dma_start(out=idxs[:, g, :], in_=t_idx.ap()[g])
            a3t = big.tile([E, VSH], c.zdt)
            nc.sync.dma_start(out=a3t[:], in_=t_a3t.ap())

            # ---- persistent work tiles ------------------------------------
            mT = [wk.tile([E, NS], F32, tag=f"mT{h}", name=f"mT{h}")
                  for h in range(3)]
            cT = [wk.tile([E, NS], F32, tag=f"cT{h}", name=f"cT{h}")
                  for h in range(3)]
            uT = wk.tile([E, Bc], F32, tag="uT")

            # ---- gather + per-chunk reductions + combines -----------------
            tile_idx = 0
            for g, gs in enumerate(c.gsizes):
                slots = gs // 128
                nxt = min(slots, c.xtiles - tile_idx)   # x-tiles this chunk
                has_q = (tile_idx + slots) > c.xtiles
                spg = nxt * c.SPT
                gs0 = tile_idx * c.SPT
                L0, L1, L2 = 0, 2 * spg, 5 * spg
                L3, Lq = 8 * spg, 9 * spg

                gt = gpool.tile([128, c.gchunk // 128, 4 * E], DT, tag="g")
                nc.gpsimd.dma_gather(
                    out_ap=gt[:, :slots, :],
                    in_ap=t_tab.ap(),
                    idxs_ap=idxs[:, g, : gs // 16],
                    num_idxs=gs,
                    num_idxs_reg=gs,
                    elem_size=4 * E,
                )
                Pg = psG.tile([128, 512], F32, space="PSUM", tag="Pg")
                for sl in range(slots):
                    t = tile_idx
                    tile_idx += 1
                    if t < c.xtiles:
                        ls0 = (t * c.SPT) - gs0
                        G0 = gt[:, sl, 0 * E:1 * E]
                        G1 = gt[:, sl, 1 * E:2 * E]
                        G2 = gt[:, sl, 2 * E:3 * E]
                        G3 = gt[:, sl, 3 * E:4 * E]
                        nc.tensor.matmul(
                            out=Pg[:, L0 + 2 * ls0: L0 + 2 * ls0 + 2 * c.SPT],
                            lhsT=G0, rhs=wab[:], start=True, stop=True)
                        nc.tensor.matmul(
                            out=Pg[:, L1 + 3 * ls0: L1 + 3 * ls0 + 3 * c.SPT],
                            lhsT=G1, rhs=wabc[:], start=True, stop=True)
                        nc.tensor.matmul(
                            out=Pg[:, L2 + 3 * ls0: L2 + 3 * ls0 + 3 * c.SPT],
                            lhsT=G2, rhs=wabc[:], start=True, stop=True)
                        nc.tensor.matmul(
                            out=Pg[:, L3 + ls0: L3 + ls0 + c.SPT],
                            lhsT=G3, rhs=wc_t[:], start=True, stop=True)
                    else:
                        nc.tensor.matmul(
                            out=Pg[:, Lq: Lq + Bc],
                            lhsT=gt[:, sl, 0 * E:1 * E], rhs=wq[:],
                            start=True, stop=True)

                # per-chunk combines (psum -> sbuf slices)
                pap = Pg[:]
                pdim = pap.ap[0]

                def pv(base, gw, off, n=spg):
                    return bass.AP(pap.tensor, pap.offset + base + off,
                                   [pdim, (gw, n)])

                if spg > 0:
                    for h, (base, gw) in enumerate(
                            [(L0, 2), (L1, 3), (L2, 3)]):
                        msl = mT[h][:, gs0:gs0 + spg]
                        nc.vector.tensor_scalar(
                            out=msl, in0=pv(base, gw, 1), scalar1=kp[:],
                            scalar2=None, op0=ALU.mult)
                        nc.vector.tensor_tensor(
                            out=msl, in0=msl, in1=pv(base, gw, 0),
                            op=ALU.add)
                    for h, (base, gw, off) in enumerate(
                            [(L1, 3, 2), (L2, 3, 2), (L3, 1, 0)]):
                        nc.vector.tensor_copy(cT[h][:, gs0:gs0 + spg],
                                              pv(base, gw, off))
                if has_q:
                    nc.vector.tensor_copy(uT[:], Pg[:, Lq: Lq + Bc])

            # ---- add temporal encodings (broadcast over batches) ----------
            def bcast_ts(tile_ap):
                return bass.AP(tile_ap.tensor, tile_ap.offset,
                               [tile_ap.ap[0], (0, Bc), tile_ap.ap[1]])

            tat_b = bcast_ts(tat[:])
            tct_b = bcast_ts(tct[:])
            for h in range(3):
                m3 = mT[h][:].rearrange("e (b s) -> e b s", b=Bc)
                nc.vector.tensor_tensor(out=m3, in0=m3, in1=tat_b, op=ALU.add)
                c3 = cT[h][:].rearrange("e (b s) -> e b s", b=Bc)
                nc.vector.tensor_tensor(out=c3, in0=c3, in1=tct_b, op=ALU.add)

            # ---- hops ------------------------------------------------------
            nchk = (NS + 127) // 128
            for h in range(3):
                sc_ps = psH.tile([Bc, NS], F32, space="PSUM", tag="sc")
                nc.tensor.matmul(out=sc_ps[:], lhsT=uT[:], rhs=mT[h][:],
                                 start=True, stop=True)
                sc = hp.tile([Bc, NS], F32, tag="sc_sb")
                nc.vector.tensor_tensor(out=sc[:], in0=sc_ps[:], in1=mask[:],
                                        op=ALU.add)
                ngmx = hp.tile([Bc, 1], F32, tag="ngmx")
                nc.vector.tensor_reduce(out=ngmx[:], in_=sc[:], axis=AX.X,
                                        op=ALU.max, negate=True)
                ex = hp.tile([Bc, NS], F32, tag="ex")
                zsum = hp.tile([Bc, 1], F32, tag="zsum")
                nc.scalar.activation(out=ex[:], in_=sc[:], func=AF.Exp,
                                     bias=ngmx[:], scale=1.0,
                                     accum_out=zsum[:])
                rz = hp.tile([Bc, 1], F32, tag="rz")
                nc.vector.reciprocal(out=rz[:], in_=zsum[:])
                nc.vector.tensor_scalar(out=ex[:], in0=ex[:], scalar1=rz[:],
                                        scalar2=None, op0=ALU.mult)
                up_ps = psH.tile([E, Bc], F32, space="PSUM", tag="up")
                for k in range(nchk):
                    w = min(128, NS - k * 128)
                    pt_ps = psH.tile([128, Bc], F32, space="PSUM", tag="ptp")
                    nc.tensor.transpose(out=pt_ps[:w, :],
                                        in_=ex[:, k * 128:k * 128 + w],
                                        identity=idq[:])
                    pt = hp.tile([128, Bc], DT, tag="pt")
                    nc.vector.tensor_copy(pt[:w, :], pt_ps[:w, :])
                    cn_ps = psH.tile([128, E], F32, space="PSUM", tag="cnp")
                    nc.tensor.transpose(out=cn_ps[:w, :],
                                        in_=cT[h][:, k * 128:k * 128 + w],
                                        identity=id128[:])
                    cn = hp.tile([128, E], DT, tag="cn")
                    nc.vector.tensor_copy(cn[:w, :], cn_ps[:w, :])
                    nc.tensor.matmul(out=up_ps[:], lhsT=cn[:w, :],
                                     rhs=pt[:w, :], start=(k == 0),
                                     stop=(k == nchk - 1))
                un = wk.tile([E, Bc], F32, tag=f"uT{h + 1}")
                nc.vector.tensor_tensor(out=un[:], in0=up_ps[:], in1=uT[:],
                                        op=ALU.add)
                uT = un

            # ---- AllGather u ----------------------------------------------
            ub_in = dram.tile([E, Bc], F32)
            ub_out = dram.tile([c.ncore * E, Bc], F32)
            nc.gpsimd.dma_start(ub_in[:], uT[:])
            nc.gpsimd.collective_compute(
                "AllGather", ALU.bypass,
                replica_groups=[list(range(c.ncore))],
                ins=[ub_in.opt()], outs=[ub_out.opt()],
            )
            uTf = wk.tile([E, c.ncore, Bc], F32, tag="uTf")
            src = bass.AP(ub_out[:].tensor, ub_out[:].offset,
                          [(Bc, E), (E * Bc, c.ncore), (1, Bc)])
            nc.sync.dma_start(out=uTf[:], in_=src)
            nc.sync.dma_start(out=t_du.ap(),
                              in_=uTf[:].rearrange("e c b -> e (c b)"))
            uz = wk.tile([E, c.B], c.zdt, tag="uz")
            nc.vector.tensor_copy(uz[:], uTf[:].rearrange("e c b -> e (c b)"))

            # ---- logits + log_softmax -------------------------------------
            zbuf = big.tile([c.B, VSH], F32)
            nzc = (VSH + 511) // 512
            sums = wk.tile([c.B, nzc], F32, tag="sums")
            for k in range(nzc):
                w = min(512, VSH - k * 512)
                zps = psZ.tile([c.B, 512], F32, space="PSUM", tag="zps")
                nc.tensor.matmul(out=zps[:, :w], lhsT=uz[:],
                                 rhs=a3t[:, k * 512:k * 512 + w],
                                 start=True, stop=True)
                nc.vector.tensor_copy(zbuf[:, k * 512:k * 512 + w],
                                      zps[:, :w])
                esc = hp.tile([c.B, 512], F32, tag="esc")
                nc.scalar.activation(out=esc[:, :w], in_=zps[:, :w],
                                     func=AF.Exp, accum_out=sums[:, k:k + 1])
            slc = wk.tile([c.B, 1], F32, tag="slc")
            nc.vector.tensor_reduce(out=slc[:], in_=sums[:], axis=AX.X,
                                    op=ALU.add)
            sb_in = dram.tile([c.B, 1], F32)
            sb_out = dram.tile([c.B, 1], F32)
            nc.gpsimd.dma_start(sb_in[:], slc[:])
            nc.gpsimd.collective_compute(
                "AllReduce", ALU.add,
                replica_groups=[list(range(c.ncore))],
                ins=[sb_in.opt()], outs=[sb_out.opt()],
            )
            st = wk.tile([c.B, 1], F32, tag="st")
            nc.sync.dma_start(out=st[:], in_=sb_out[:])
            lse = wk.tile([c.B, 1], F32, tag="lse")
            nc.scalar.activation(out=lse[:], in_=st[:], func=AF.Ln)
            nc.vector.tensor_scalar(out=zbuf[:], in0=zbuf[:], scalar1=lse[:],
                                    scalar2=None, op0=ALU.subtract)
            nc.sync.dma_start(out=t_o.ap(), in_=zbuf[:])

    nc.compile()
    return nc


def host_prep(cfg, x, q, A, TA, TC):
    c = cfg
    E, J, S = c.E, c.J, c.S
    x = np.asarray(x).astype(np.int64)
    q = np.asarray(q).astype(np.int64)
    A = np.asarray(A, dtype=np.float32)
    TA = np.asarray(TA, dtype=np.float32)
    TC = np.asarray(TC, dtype=np.float32)

    tabI = np.ascontiguousarray(A.transpose(1, 0, 2).reshape(c.V, 4 * E))
    tabI = tabI.astype(c.npdt)
    a3tF = np.ascontiguousarray(A[3].T)  # [E, V] f32

    j = np.arange(1, J + 1, dtype=np.float32)
    av = 1.0 - j / J
    bv = 2.0 * j / J - 1.0
    sp = np.arange(128) // J
    jj = np.arange(128) % J
    wab = np.zeros((128, 2 * c.SPT), np.float32)
    wabc = np.zeros((128, 3 * c.SPT), np.float32)
    wc = np.zeros((128, c.SPT), np.float32)
    for p in range(128):
        wab[p, 2 * sp[p] + 0] = av[jj[p]]
        wab[p, 2 * sp[p] + 1] = bv[jj[p]]
        wabc[p, 3 * sp[p] + 0] = av[jj[p]]
        wabc[p, 3 * sp[p] + 1] = bv[jj[p]]
        wabc[p, 3 * sp[p] + 2] = 1.0
        wc[p, sp[p]] = 1.0
    wq = np.zeros((128, c.Bc), np.float32)
    for p in range(128):
        wq[p, p // c.QW] = 1.0

    tat = np.ascontiguousarray(TA[0, :S, :].T)
    tct = np.ascontiguousarray(TC[0, :S, :].T)
    kp = ((np.arange(E, dtype=np.float32) + 1.0) / E).reshape(E, 1)
    mask = np.full((c.Bc, c.NS), NEG, np.float32)
    for b in range(c.Bc):
        mask[b, b * S:(b + 1) * S] = 0.0
    id128 = np.eye(128, dtype=np.float32)
    idq = np.eye(c.Bc, dtype=np.float32)

    common = {
        "wab": wab.astype(c.npdt), "wabc": wabc.astype(c.npdt),
        "wc": wc.astype(c.npdt), "wq": wq.astype(c.npdt),
        "tat": tat, "tct": tct, "kp": kp, "mask": mask,
        "id128": id128, "idq": idq,
    }

    nch = len(c.gsizes)
    in_maps = []
    for cc in range(c.ncore):
        xc = x[cc * c.Bc:(cc + 1) * c.Bc].reshape(-1)
        qc = q[cc * c.Bc:(cc + 1) * c.Bc].reshape(-1)
        xq = np.concatenate([xc, qc])
        uniq, rel = np.unique(xq, return_inverse=True)
        assert len(uniq) <= c.ucap, (len(uniq), c.ucap)
        tabc = np.zeros((c.ucap, 4 * E), c.npdt)
        tabc[:len(uniq)] = tabI[uniq]
        rel = rel.astype(np.int16)
        idx = np.zeros((nch, 128, c.gchunk // 16), np.int16)
        off = 0
        for g, gs in enumerate(c.gsizes):
            v = rel[off:off + gs]
            off += gs
            wrapped = v.reshape(-1, 16).T
            idx[g, :, : gs // 16] = np.tile(wrapped, (8, 1))
        a3c = np.ascontiguousarray(
            a3tF[:, cc * c.VSH:(cc + 1) * c.VSH]).astype(c.znp)
        m = dict(common)
        m.update({"tabc": tabc, "idx": idx, "a3t": a3c})
        in_maps.append(m)
    return in_maps


_CACHE = {}


def _get_module(cfg):
    k = cfg.key()
    if k not in _CACHE:
        _CACHE[k] = build_module(cfg)
    return _CACHE[k]


def run(cfg, inputs, trace=False):
    nc = _get_module(cfg)
    in_maps = host_prep(cfg, inputs["x"], inputs["q"], inputs["A"],
                        inputs["TA"], inputs["TC"])
    res = bass_utils.run_bass_kernel_spmd(
        nc, in_maps, core_ids=list(range(cfg.ncore)), trace=trace)
    out = np.concatenate([res.results[cc]["o"] for cc in range(cfg.ncore)],
                         axis=1)
    return out, res


def kernel(**inputs) -> np.ndarray:
    cfg = Cfg()
    out, _ = run(cfg, inputs, trace=False)
    return out



# revision 3
# speedup vs baseline: 1.2503x; 1.2503x over previous
"""MemNN (embedding_lookup) Trainium2 Bass kernel.

Strategy (8 NeuronCores, one NEFF, SPMD):
  - Data-parallel hops: batch dim sharded 8 ways (8 batches/core).
  - Host packs the 4 embedding tables interleaved per vocab row
    ([A0|A1|A2|A3][v], bf16) and, per core, compacts it to the core's
    unique vocab rows so indices fit dma_gather's int16 (~22.6K < 32767).
  - dma_gather streams all (story + query) embedding rows; PE matmuls
    with fixed block weights reduce each 128-row tile into per-sentence
    partial sums (one PSUM bank per gather chunk). Position encoding is
    rank-2 separable:
        pe[j,d] = a(j) + b(j) * k'(d),  a=1-j/J, b=2j/J-1, k'=d/D
    so m = S_a + k' * S_b needs only two weighted sums per sentence.
  - 3 attention hops on PE/DVE/ACT in transposed [d, sentence] layout.
  - AllGather u across cores; vocab-sharded logits z = u @ A3^T using a
    host-pre-transposed A3 shard; log_softmax via AllReduce of exp-sums.
"""

import numpy as np
import ml_dtypes

import concourse.bass as bass
import concourse.mybir as mybir
import concourse.tile as tile
from concourse import bacc
import concourse.bass_utils as bass_utils

F32 = mybir.dt.float32
AF = mybir.ActivationFunctionType
ALU = mybir.AluOpType
AX = mybir.AxisListType

NEG = -1e30


class Cfg:
    def __init__(self, ncore=8, B=64, S=50, J=64, QW=16, V=100000, E=128,
                 ucap=24576, gchunk=1024, use_bf16=True, z_f32=False):
        self.ncore, self.B, self.S, self.J, self.QW = ncore, B, S, J, QW
        self.V, self.E, self.ucap, self.gchunk = V, E, ucap, gchunk
        self.use_bf16, self.z_f32 = use_bf16, z_f32
        self.Bc = B // ncore
        self.NS = self.Bc * S              # sentences per core
        self.NX = self.NS * J              # story rows per core
        self.NQ = self.Bc * QW             # query rows per core
        assert self.NQ == 128 and self.NX % 128 == 0
        self.NPOS = self.NX + self.NQ
        self.xtiles = self.NX // 128
        self.SPT = 128 // J                # sentences per 128-row tile
        assert 128 % J == 0
        self.VSH = V // ncore
        sizes = []
        rem = self.NPOS
        while rem > 0:
            s = min(gchunk, rem)
            sizes.append(s)
            rem -= s
        assert all(s % 128 == 0 for s in sizes)
        self.gsizes = sizes
        self.DT = mybir.dt.bfloat16 if use_bf16 else mybir.dt.float32
        self.npdt = ml_dtypes.bfloat16 if use_bf16 else np.float32
        self.zdt = F32 if z_f32 else self.DT
        self.znp = np.float32 if z_f32 else self.npdt

    def key(self):
        return (self.ncore, self.B, self.S, self.J, self.QW, self.V, self.E,
                self.ucap, self.gchunk, self.use_bf16, self.z_f32)


def build_module(cfg):
    c = cfg
    E, NS, Bc, VSH, S = c.E, c.NS, c.Bc, c.VSH, c.S
    DT = c.DT
    nc = bacc.Bacc("TRN2", target_bir_lowering=False, debug=False,
                   num_devices=c.ncore, num_swdge_queues=4)

    t_tab = nc.dram_tensor("tabc", [c.ucap, 4 * E], DT, kind="ExternalInput")
    nch = len(c.gsizes)
    t_idx = nc.dram_tensor("idx", [nch, 128, c.gchunk // 16], mybir.dt.int16,
                           kind="ExternalInput")
    t_a3t = nc.dram_tensor("a3t", [E, VSH], c.zdt, kind="ExternalInput")
    t_wab = nc.dram_tensor("wab", [128, 2 * c.SPT], DT, kind="ExternalInput")
    t_wabc = nc.dram_tensor("wabc", [128, 3 * c.SPT], DT, kind="ExternalInput")
    t_wc = nc.dram_tensor("wc", [128, c.SPT], DT, kind="ExternalInput")
    t_wq = nc.dram_tensor("wq", [128, Bc], DT, kind="ExternalInput")
    t_tat = nc.dram_tensor("tat", [E, S], F32, kind="ExternalInput")
    t_tct = nc.dram_tensor("tct", [E, S], F32, kind="ExternalInput")
    t_kp = nc.dram_tensor("kp", [E, 1], F32, kind="ExternalInput")
    t_mask = nc.dram_tensor("mask", [Bc, NS], F32, kind="ExternalInput")
    t_id128 = nc.dram_tensor("id128", [128, 128], F32, kind="ExternalInput")
    t_idq = nc.dram_tensor("idq", [Bc, Bc], F32, kind="ExternalInput")

    t_o = nc.dram_tensor("o", [c.B, VSH], F32, kind="ExternalOutput")
    t_du = nc.dram_tensor("du", [E, c.B], F32, kind="ExternalOutput")

    with tile.TileContext(nc) as tc:
        with tc.tile_pool(name="const", bufs=1) as cpool, \
             tc.tile_pool(name="gp", bufs=3) as gpool, \
             tc.tile_pool(name="wk", bufs=1) as wk, \
             tc.tile_pool(name="hp", bufs=2) as hp, \
             tc.tile_pool(name="big", bufs=1) as big, \
             tc.tile_pool(name="psG", bufs=2, space="PSUM") as psG, \
             tc.tile_pool(name="psH", bufs=1, space="PSUM") as psH, \
             tc.tile_pool(name="psZ", bufs=2, space="PSUM") as psZ, \
             tc.tile_pool(name="dram", bufs=1, space="DRAM") as dram:

            # ---- constant loads -------------------------------------------
            wab = cpool.tile([128, 2 * c.SPT], DT)
            nc.sync.dma_start(out=wab[:], in_=t_wab.ap())
            wabc = cpool.tile([128, 3 * c.SPT], DT)
            nc.sync.dma_start(out=wabc[:], in_=t_wabc.ap())
            wc_t = cpool.tile([128, c.SPT], DT)
            nc.sync.dma_start(out=wc_t[:], in_=t_wc.ap())
            wq = cpool.tile([128, Bc], DT)
            nc.sync.dma_start(out=wq[:], in_=t_wq.ap())
            tat = cpool.tile([E, S], F32)
            nc.sync.dma_start(out=tat[:], in_=t_tat.ap())
            tct = cpool.tile([E, S], F32)
            nc.sync.dma_start(out=tct[:], in_=t_tct.ap())
            kp = cpool.tile([E, 1], F32)
            nc.sync.dma_start(out=kp[:], in_=t_kp.ap())
            mask = cpool.tile([Bc, NS], F32)
            nc.sync.dma_start(out=mask[:], in_=t_mask.ap())
            id128 = cpool.tile([128, 128], F32)
            nc.sync.dma_start(out=id128[:], in_=t_id128.ap())
            idq = cpool.tile([Bc, Bc], F32)
            nc.sync.dma_start(out=idq[:], in_=t_idq.ap())
            idxs = cpool.tile([128, nch, c.gchunk // 16], mybir.dt.int16)
            for g in range(nch):
                nc.sync.dma_start(out=idxs[:, g, :], in_=t_idx.ap()[g])
            a3t = big.tile([E, VSH], c.zdt)
            nc.sync.dma_start(out=a3t[:], in_=t_a3t.ap())

            # ---- persistent work tiles ------------------------------------
            mT = [wk.tile([E, NS], F32, tag=f"mT{h}", name=f"mT{h}")
                  for h in range(3)]
            cT = [wk.tile([E, NS], F32, tag=f"cT{h}", name=f"cT{h}")
                  for h in range(3)]
            uT = wk.tile([E, Bc], F32, tag="uT")

            # ---- gather + per-chunk reductions + combines -----------------
            tile_idx = 0
            for g, gs in enumerate(c.gsizes):
                slots = gs // 128
                nxt = min(slots, c.xtiles - tile_idx)   # x-tiles this chunk
                has_q = (tile_idx + slots) > c.xtiles
                spg = nxt * c.SPT
                gs0 = tile_idx * c.SPT
                L0, L1, L2 = 0, 2 * spg, 5 * spg
                L3, Lq = 8 * spg, 9 * spg

                gt = gpool.tile([128, c.gchunk // 128, 4 * E], DT, tag="g")
                nc.gpsimd.dma_gather(
                    out_ap=gt[:, :slots, :],
                    in_ap=t_tab.ap(),
                    idxs_ap=idxs[:, g, : gs // 16],
                    num_idxs=gs,
                    num_idxs_reg=gs,
                    elem_size=4 * E,
                    queue_num=g % 4,
                )
                Pg = psG.tile([128, 512], F32, space="PSUM", tag="Pg")
                for sl in range(slots):
                    t = tile_idx
                    tile_idx += 1
                    if t < c.xtiles:
                        ls0 = (t * c.SPT) - gs0
                        G0 = gt[:, sl, 0 * E:1 * E]
                        G1 = gt[:, sl, 1 * E:2 * E]
                        G2 = gt[:, sl, 2 * E:3 * E]
                        G3 = gt[:, sl, 3 * E:4 * E]
                        nc.tensor.matmul(
                            out=Pg[:, L0 + 2 * ls0: L0 + 2 * ls0 + 2 * c.SPT],
                            lhsT=G0, rhs=wab[:], start=True, stop=True)
                        nc.tensor.matmul(
                            out=Pg[:, L1 + 3 * ls0: L1 + 3 * ls0 + 3 * c.SPT],
                            lhsT=G1, rhs=wabc[:], start=True, stop=True)
                        nc.tensor.matmul(
                            out=Pg[:, L2 + 3 * ls0: L2 + 3 * ls0 + 3 * c.SPT],
                            lhsT=G2, rhs=wabc[:], start=True, stop=True)
                        nc.tensor.matmul(
                            out=Pg[:, L3 + ls0: L3 + ls0 + c.SPT],
                            lhsT=G3, rhs=wc_t[:], start=True, stop=True)
                    else:
                        nc.tensor.matmul(
                            out=Pg[:, Lq: Lq + Bc],
                            lhsT=gt[:, sl, 0 * E:1 * E], rhs=wq[:],
                            start=True, stop=True)

                # per-chunk combines (psum -> sbuf slices)
                pap = Pg[:]
                pdim = pap.ap[0]

                def pv(base, gw, off, n=spg):
                    return bass.AP(pap.tensor, pap.offset + base + off,
                                   [pdim, (gw, n)])

                if spg > 0:
                    for h, (base, gw) in enumerate(
                            [(L0, 2), (L1, 3), (L2, 3)]):
                        msl = mT[h][:, gs0:gs0 + spg]
                        nc.vector.tensor_scalar(
                            out=msl, in0=pv(base, gw, 1), scalar1=kp[:],
                            scalar2=None, op0=ALU.mult)
                        nc.vector.tensor_tensor(
                            out=msl, in0=msl, in1=pv(base, gw, 0),
                            op=ALU.add)
                    for h, (base, gw, off) in enumerate(
                            [(L1, 3, 2), (L2, 3, 2), (L3, 1, 0)]):
                        nc.vector.tensor_copy(cT[h][:, gs0:gs0 + spg],
                                              pv(base, gw, off))
                if has_q:
                    nc.vector.tensor_copy(uT[:], Pg[:, Lq: Lq + Bc])

            # ---- add temporal encodings (broadcast over batches) ----------
            def bcast_ts(tile_ap):
                return bass.AP(tile_ap.tensor, tile_ap.offset,
                               [tile_ap.ap[0], (0, Bc), tile_ap.ap[1]])

            tat_b = bcast_ts(tat[:])
            tct_b = bcast_ts(tct[:])
            for h in range(3):
                m3 = mT[h][:].rearrange("e (b s) -> e b s", b=Bc)
                nc.vector.tensor_tensor(out=m3, in0=m3, in1=tat_b, op=ALU.add)
                c3 = cT[h][:].rearrange("e (b s) -> e b s", b=Bc)
                nc.vector.tensor_tensor(out=c3, in0=c3, in1=tct_b, op=ALU.add)

            # ---- hops ------------------------------------------------------
            nchk = (NS + 127) // 128
            for h in range(3):
                sc_ps = psH.tile([Bc, NS], F32, space="PSUM", tag="sc")
                nc.tensor.matmul(out=sc_ps[:], lhsT=uT[:], rhs=mT[h][:],
                                 start=True, stop=True)
                sc = hp.tile([Bc, NS], F32, tag="sc_sb")
                nc.vector.tensor_tensor(out=sc[:], in0=sc_ps[:], in1=mask[:],
                                        op=ALU.add)
                ngmx = hp.tile([Bc, 1], F32, tag="ngmx")
                nc.vector.tensor_reduce(out=ngmx[:], in_=sc[:], axis=AX.X,
                                        op=ALU.max, negate=True)
                ex = hp.tile([Bc, NS], F32, tag="ex")
                zsum = hp.tile([Bc, 1], F32, tag="zsum")
                nc.scalar.activation(out=ex[:], in_=sc[:], func=AF.Exp,
                                     bias=ngmx[:], scale=1.0,
                                     accum_out=zsum[:])
                rz = hp.tile([Bc, 1], F32, tag="rz")
                nc.vector.reciprocal(out=rz[:], in_=zsum[:])
                nc.vector.tensor_scalar(out=ex[:], in0=ex[:], scalar1=rz[:],
                                        scalar2=None, op0=ALU.mult)
                up_ps = psH.tile([E, Bc], F32, space="PSUM", tag="up")
                for k in range(nchk):
                    w = min(128, NS - k * 128)
                    pt_ps = psH.tile([128, Bc], F32, space="PSUM", tag="ptp")
                    nc.tensor.transpose(out=pt_ps[:w, :],
                                        in_=ex[:, k * 128:k * 128 + w],
                                        identity=idq[:])
                    pt = hp.tile([128, Bc], DT, tag="pt")
                    nc.vector.tensor_copy(pt[:w, :], pt_ps[:w, :])
                    cn_ps = psH.tile([128, E], F32, space="PSUM", tag="cnp")
                    nc.tensor.transpose(out=cn_ps[:w, :],
                                        in_=cT[h][:, k * 128:k * 128 + w],
                                        identity=id128[:])
                    cn = hp.tile([128, E], DT, tag="cn")
                    nc.vector.tensor_copy(cn[:w, :], cn_ps[:w, :])
                    nc.tensor.matmul(out=up_ps[:], lhsT=cn[:w, :],
                                     rhs=pt[:w, :], start=(k == 0),
                                     stop=(k == nchk - 1))
                un = wk.tile([E, Bc], F32, tag=f"uT{h + 1}")
                nc.vector.tensor_tensor(out=un[:], in0=up_ps[:], in1=uT[:],
                                        op=ALU.add)
                uT = un

            # ---- AllGather u ----------------------------------------------
            ub_in = dram.tile([E, Bc], F32)
            ub_out = dram.tile([c.ncore * E, Bc], F32)
            nc.gpsimd.dma_start(ub_in[:], uT[:])
            nc.gpsimd.collective_compute(
                "AllGather", ALU.bypass,
                replica_groups=[list(range(c.ncore))],
                ins=[ub_in.opt()], outs=[ub_out.opt()],
            )
            uTf = wk.tile([E, c.ncore, Bc], F32, tag="uTf")
            src = bass.AP(ub_out[:].tensor, ub_out[:].offset,
                          [(Bc, E), (E * Bc, c.ncore), (1, Bc)])
            nc.sync.dma_start(out=uTf[:], in_=src)
            nc.sync.dma_start(out=t_du.ap(),
                              in_=uTf[:].rearrange("e c b -> e (c b)"))
            uz = wk.tile([E, c.B], c.zdt, tag="uz")
            nc.vector.tensor_copy(uz[:], uTf[:].rearrange("e c b -> e (c b)"))

            # ---- logits + log_softmax -------------------------------------
            zbuf = big.tile([c.B, VSH], F32)
            nzc = (VSH + 511) // 512
            sums = wk.tile([c.B, nzc], F32, tag="sums")
            for k in range(nzc):
                w = min(512, VSH - k * 512)
                zps = psZ.tile([c.B, 512], F32, space="PSUM", tag="zps")
                nc.tensor.matmul(out=zps[:, :w], lhsT=uz[:],
                                 rhs=a3t[:, k * 512:k * 512 + w],
                                 start=True, stop=True)
                nc.vector.tensor_copy(zbuf[:, k * 512:k * 512 + w],
                                      zps[:, :w])
                esc = hp.tile([c.B, 512], F32, tag="esc")
                nc.scalar.activation(out=esc[:, :w], in_=zps[:, :w],
                                     func=AF.Exp, accum_out=sums[:, k:k + 1])
            slc = wk.tile([c.B, 1], F32, tag="slc")
            nc.vector.tensor_reduce(out=slc[:], in_=sums[:], axis=AX.X,
                                    op=ALU.add)
            sb_in = dram.tile([c.B, 1], F32)
            sb_out = dram.tile([c.B, 1], F32)
            nc.gpsimd.dma_start(sb_in[:], slc[:])
            nc.gpsimd.collective_compute(
                "AllReduce", ALU.add,
                replica_groups=[list(range(c.ncore))],
                ins=[sb_in.opt()], outs=[sb_out.opt()],
            )
            st = wk.tile([c.B, 1], F32, tag="st")
            nc.sync.dma_start(out=st[:], in_=sb_out[:])
            lse = wk.tile([c.B, 1], F32, tag="lse")
            nc.scalar.activation(out=lse[:], in_=st[:], func=AF.Ln)
            nc.vector.tensor_scalar(out=zbuf[:], in0=zbuf[:], scalar1=lse[:],
                                    scalar2=None, op0=ALU.subtract)
            nc.sync.dma_start(out=t_o.ap(), in_=zbuf[:])

    nc.compile()
    return nc


def host_prep(cfg, x, q, A, TA, TC):
    c = cfg
    E, J, S = c.E, c.J, c.S
    x = np.asarray(x).astype(np.int64)
    q = np.asarray(q).astype(np.int64)
    A = np.asarray(A, dtype=np.float32)
    TA = np.asarray(TA, dtype=np.float32)
    TC = np.asarray(TC, dtype=np.float32)

    tabI = np.ascontiguousarray(A.transpose(1, 0, 2).reshape(c.V, 4 * E))
    tabI = tabI.astype(c.npdt)
    a3tF = np.ascontiguousarray(A[3].T)  # [E, V] f32

    j = np.arange(1, J + 1, dtype=np.float32)
    av = 1.0 - j / J
    bv = 2.0 * j / J - 1.0
    sp = np.arange(128) // J
    jj = np.arange(128) % J
    wab = np.zeros((128, 2 * c.SPT), np.float32)
    wabc = np.zeros((128, 3 * c.SPT), np.float32)
    wc = np.zeros((128, c.SPT), np.float32)
    for p in range(128):
        wab[p, 2 * sp[p] + 0] = av[jj[p]]
        wab[p, 2 * sp[p] + 1] = bv[jj[p]]
        wabc[p, 3 * sp[p] + 0] = av[jj[p]]
        wabc[p, 3 * sp[p] + 1] = bv[jj[p]]
        wabc[p, 3 * sp[p] + 2] = 1.0
        wc[p, sp[p]] = 1.0
    wq = np.zeros((128, c.Bc), np.float32)
    for p in range(128):
        wq[p, p // c.QW] = 1.0

    tat = np.ascontiguousarray(TA[0, :S, :].T)
    tct = np.ascontiguousarray(TC[0, :S, :].T)
    kp = ((np.arange(E, dtype=np.float32) + 1.0) / E).reshape(E, 1)
    mask = np.full((c.Bc, c.NS), NEG, np.float32)
    for b in range(c.Bc):
        mask[b, b * S:(b + 1) * S] = 0.0
    id128 = np.eye(128, dtype=np.float32)
    idq = np.eye(c.Bc, dtype=np.float32)

    common = {
        "wab": wab.astype(c.npdt), "wabc": wabc.astype(c.npdt),
        "wc": wc.astype(c.npdt), "wq": wq.astype(c.npdt),
        "tat": tat, "tct": tct, "kp": kp, "mask": mask,
        "id128": id128, "idq": idq,
    }

    nch = len(c.gsizes)
    in_maps = []
    for cc in range(c.ncore):
        xc = x[cc * c.Bc:(cc + 1) * c.Bc].reshape(-1)
        qc = q[cc * c.Bc:(cc + 1) * c.Bc].reshape(-1)
        xq = np.concatenate([xc, qc])
        uniq, rel = np.unique(xq, return_inverse=True)
        assert len(uniq) <= c.ucap, (len(uniq), c.ucap)
        tabc = np.zeros((c.ucap, 4 * E), c.npdt)
        tabc[:len(uniq)] = tabI[uniq]
        rel = rel.astype(np.int16)
        idx = np.zeros((nch, 128, c.gchunk // 16), np.int16)
        off = 0
        for g, gs in enumerate(c.gsizes):
            v = rel[off:off + gs]
            off += gs
            wrapped = v.reshape(-1, 16).T
            idx[g, :, : gs // 16] = np.tile(wrapped, (8, 1))
        a3c = np.ascontiguousarray(
            a3tF[:, cc * c.VSH:(cc + 1) * c.VSH]).astype(c.znp)
        m = dict(common)
        m.update({"tabc": tabc, "idx": idx, "a3t": a3c})
        in_maps.append(m)
    return in_maps


_CACHE = {}


def _get_module(cfg):
    k = cfg.key()
    if k not in _CACHE:
        _CACHE[k] = build_module(cfg)
    return _CACHE[k]


def run(cfg, inputs, trace=False):
    nc = _get_module(cfg)
    in_maps = host_prep(cfg, inputs["x"], inputs["q"], inputs["A"],
                        inputs["TA"], inputs["TC"])
    res = bass_utils.run_bass_kernel_spmd(
        nc, in_maps, core_ids=list(range(cfg.ncore)), trace=trace)
    out = np.concatenate([res.results[cc]["o"] for cc in range(cfg.ncore)],
                         axis=1)
    return out, res


def kernel(**inputs) -> np.ndarray:
    cfg = Cfg()
    out, _ = run(cfg, inputs, trace=False)
    return out

